# revision 1
# baseline (speedup 1.0000x reference)
"""Dual-pixel depth-merge (forward splat) kernel for Trainium2, 8 NeuronCores.

Math: for integer pixel grid x, the reference computes pos = fl(x +- depth)
(f32-rounded), x0 = floor(pos), f = pos - x0. Define the per-view fractional
offsets
    v_l[i] = fl(i + depth[i]) - i   (exact f32 subtraction, in [0, 8])
    v_r[i] = i - fl(i - depth[i])   (exact f32 subtraction, in [0, 8])
Then each view's splat is a 9-tap shifted weighted sum with hat weights
    Wl_d = relu(1 - |v_l - d|),  Wr_d = relu(1 - |v_r - d|),  d = 0..8:
    count_l[j] = sum_d Wl_d[j-d]      acc_l[c,j] = sum_d (Wl_d*img_c)[j-d]
    count_r[j] = sum_d Wr_d[j+d]      acc_r[c,j] = sum_d (Wr_d*img_c)[j+d]
    left = acc_l / max(count_l, eps)  right = acc_r / max(count_r, eps)
This reproduces the reference's weights bit-for-bit (matching its f32
rounding of x+-depth), so count==0 happens exactly where the reference's
does — and there acc==0 too, making the eps-divide equal the reference's
where(count==0, 1, count).

Sharding: pure data parallel over h (the scatter is along w only) — core m
takes h rows [m*128, (m+1)*128) for all batches. No halo, no communication.
"""

import numpy as np

import concourse.bacc as bacc
import concourse.bass as bass
import concourse.mybir as mybir
import concourse.tile as tile
from concourse.bass_utils import run_bass_kernel_spmd

B, C, H, W = 4, 3, 1024, 1024
NCORES = 8
HS = H // NCORES  # 128 h-rows per core
NTAP = 9
F32 = mybir.dt.float32
EPS = 1e-20

_MAX = mybir.AluOpType.max
_ADD = mybir.AluOpType.add
_SUB = mybir.AluOpType.subtract
_RELU = mybir.ActivationFunctionType.Relu
_ABS = mybir.ActivationFunctionType.Abs
_IDENT = mybir.ActivationFunctionType.Identity


def _bcast_c(ap):
    """View a [HS, W] tile as [HS, C, W] by repeating along a step-0 dim."""
    a = ap.ap
    return bass.AP(tensor=ap.tensor, offset=ap.offset, ap=[list(a[0]), [0, C], list(a[1])])


CFG = {
    "io": 2, "w": 4, "v": 2, "t": 2, "p": 3, "acc": 2, "accn": 2,
    # Column split: DVE handles w in [0, spl), GPSIMD handles [spl, W), for
    # products/adds (spl_p) and count sums (spl_c).
    "spl_p": 688,
    "spl_a": 656,
    "spl_c": 688,
}


def build_program(skip: frozenset = frozenset(), cfg: dict | None = None) -> bass.Bass:
    """skip: debug knob — subset of {"counts", "prods", "weights"} to omit
    (produces wrong results; used only for critical-path bisection)."""
    cfg = {**CFG, **(cfg or {})}
    nc = bacc.Bacc()
    image = nc.dram_tensor("image", [B, C, HS, W], F32, kind="ExternalInput")
    depth = nc.dram_tensor("depth", [B, HS, W], F32, kind="ExternalInput")
    left = nc.dram_tensor("left", [B, C, HS, W], F32, kind="ExternalOutput")
    right = nc.dram_tensor("right", [B, C, HS, W], F32, kind="ExternalOutput")

    with tile.TileContext(nc) as tc:
        with (
            tc.tile_pool(name="consts", bufs=1) as c_pool,
            tc.tile_pool(name="io", bufs=cfg["io"]) as io_pool,
            tc.tile_pool(name="wts", bufs=cfg["w"]) as w_pool,
            tc.tile_pool(name="voff", bufs=cfg["v"]) as v_pool,
            tc.tile_pool(name="tmp", bufs=cfg["t"]) as t_pool,
            tc.tile_pool(name="prod", bufs=cfg["p"]) as p_pool,
            tc.tile_pool(name="accs", bufs=cfg["acc"]) as acc_pool,
            tc.tile_pool(name="accn", bufs=cfg["accn"]) as accn_pool,
        ):
            # Per-tap bias constants and the column-index (iota) row.
            negd = c_pool.tile([HS, NTAP], F32, tag="negd")
            for d in range(NTAP):
                nc.vector.memset(negd[:, d : d + 1], -float(d))
            epsb = c_pool.tile([HS, 1], F32, tag="epsb")
            nc.vector.memset(epsb[:], EPS)
            iota_i = t_pool.tile([HS, W], mybir.dt.int32, tag="t")
            nc.gpsimd.iota(iota_i[:], [[1, W]], channel_multiplier=0)
            iota = c_pool.tile([HS, W], F32, tag="iota")
            nc.vector.tensor_copy(iota[:], iota_i[:])

            for b in range(B):
                dep = io_pool.tile([HS, W], F32, tag="dep")
                nc.sync.dma_start(out=dep[:], in_=depth[b])
                img = io_pool.tile([HS, C, W], F32, tag="img")
                nc.sync.dma_start(out=img[:], in_=image[b].transpose([1, 0, 2]))

                # Exact per-view fractional offsets (reproduce reference's
                # f32 rounding of x +- depth; the second subtract is exact).
                vl = v_pool.tile([HS, W], F32, tag="vl")
                vr = v_pool.tile([HS, W], F32, tag="vr")
                s = t_pool.tile([HS, W], F32, tag="s")
                nc.vector.tensor_tensor(s[:], dep[:], iota[:], _ADD)
                nc.vector.tensor_tensor(vl[:], s[:], iota[:], _SUB)
                s2 = t_pool.tile([HS, W], F32, tag="s")
                nc.gpsimd.tensor_tensor(s2[:], iota[:], dep[:], _SUB)
                nc.gpsimd.tensor_tensor(vr[:], iota[:], s2[:], _SUB)

                # Interleave the two views tap-by-tap so DVE/GPSIMD/ACT all
                # stay fed. Work is column-split: DVE takes [0, spl),
                # GPSIMD [spl, W) of every product/add/count op.
                sp = cfg["spl_p"]
                sa = cfg["spl_a"]
                sc = cfg["spl_c"]
                views = (("l", vl), ("r", vr))
                cnt_l = accn_pool.tile([HS, W], F32, tag="cl")
                cnt_r = accn_pool.tile([HS, W], F32, tag="cr")
                acc_l = acc_pool.tile([HS, C, W], F32, tag="al")
                acc_r = acc_pool.tile([HS, C, W], F32, tag="ar")
                cnts = {"l": cnt_l, "r": cnt_r}
                accs = {"l": acc_l, "r": acc_r}
                for d in range(NTAP):
                    for view, v in views:
                        cnt, acc = cnts[view], accs[view]
                        # Tap weight W_d = relu(1 - |v - d|) on the scalar engine.
                        td = t_pool.tile([HS, W], F32, tag="t")
                        nc.scalar.activation(td[:], v[:], _ABS, bias=negd[:, d : d + 1], scale=1.0)
                        wd = w_pool.tile([HS, W], F32, tag="w")
                        nc.scalar.activation(wd[:], td[:], _RELU, bias=1.0, scale=-1.0)
                        if d == 0:
                            nc.scalar.copy(cnt[:], wd[:])
                            nc.vector.tensor_mul(acc[:, :, 0:sp], _bcast_c(wd[:, 0:sp]), img[:, :, 0:sp])
                            nc.gpsimd.tensor_mul(acc[:, :, sp:W], _bcast_c(wd[:, sp:W]), img[:, :, sp:W])
                            continue
                        if "counts" not in skip:
                            if view == "l":
                                nc.vector.tensor_tensor(cnt[:, d:sc], cnt[:, d:sc], wd[:, 0 : sc - d], _ADD)
                                nc.gpsimd.tensor_tensor(cnt[:, sc:W], cnt[:, sc:W], wd[:, sc - d : W - d], _ADD)
                            else:
                                nc.vector.tensor_tensor(cnt[:, 0:sc], cnt[:, 0:sc], wd[:, d : sc + d], _ADD)
                                nc.gpsimd.tensor_tensor(cnt[:, sc : W - d], cnt[:, sc : W - d], wd[:, sc + d : W], _ADD)
                        if "prods" not in skip:
                            pd = p_pool.tile([HS, C, W], F32, tag="p")
                            nc.vector.tensor_mul(pd[:, :, 0:sp], _bcast_c(wd[:, 0:sp]), img[:, :, 0:sp])
                            nc.gpsimd.tensor_mul(pd[:, :, sp:W], _bcast_c(wd[:, sp:W]), img[:, :, sp:W])
                            if view == "l":
                                nc.vector.tensor_add(acc[:, :, d:sa], acc[:, :, d:sa], pd[:, :, 0 : sa - d])
                                nc.gpsimd.tensor_add(acc[:, :, sa:W], acc[:, :, sa:W], pd[:, :, sa - d : W - d])
                            else:
                                nc.vector.tensor_add(acc[:, :, 0:sa], acc[:, :, 0:sa], pd[:, :, d : sa + d])
                                nc.gpsimd.tensor_add(acc[:, :, sa : W - d], acc[:, :, sa : W - d], pd[:, :, sa + d : W])

                # Normalize: out = acc * (1 / max(count, eps)).
                for view, _ in views:
                    cnt, acc = cnts[view], accs[view]
                    # count >= 0 and its smallest nonzero value is ~6e-8, so
                    # count + 1e-20 is bit-identical to max(count, 1e-20) —
                    # and an add-constant runs on the idle scalar engine.
                    rc = accn_pool.tile([HS, W], F32, tag=f"rc{view}")
                    nc.scalar.activation(cnt[:], cnt[:], _IDENT, bias=epsb[:], scale=1.0)
                    nc.vector.reciprocal_approx_fast(out=rc[:], in_=cnt[:])
                    nc.vector.tensor_mul(acc[:, :, 0:sa], acc[:, :, 0:sa], _bcast_c(rc[:, 0:sa]))
                    nc.gpsimd.tensor_mul(acc[:, :, sa:W], acc[:, :, sa:W], _bcast_c(rc[:, sa:W]))

                nc.sync.dma_start(out=left[b].transpose([1, 0, 2]), in_=accs["l"][:])
                nc.sync.dma_start(out=right[b].transpose([1, 0, 2]), in_=accs["r"][:])
    nc.compile()
    return nc


_NC_CACHE = None


def _get_program():
    global _NC_CACHE
    if _NC_CACHE is None:
        _NC_CACHE = build_program()
    return _NC_CACHE


def kernel(image: np.ndarray, depth: np.ndarray):
    image = np.ascontiguousarray(image, dtype=np.float32)
    depth = np.ascontiguousarray(depth, dtype=np.float32)
    assert image.shape == (B, C, H, W) and depth.shape == (B, H, W)

    nc = _get_program()
    in_maps = []
    for m in range(NCORES):
        sl = slice(m * HS, (m + 1) * HS)
        in_maps.append(
            {
                "image": np.ascontiguousarray(image[:, :, sl, :]),
                "depth": np.ascontiguousarray(depth[:, sl, :]),
            }
        )
    # The axon-tunneled devices occasionally come up in a transient
    # unrecoverable/desynced state (e.g. poisoned by a previous failed
    # process) and recover on the next attempt — retry once before giving up.
    try:
        res = run_bass_kernel_spmd(nc, in_maps, core_ids=list(range(NCORES)))
    except Exception:
        import time as _time

        _time.sleep(5.0)
        res = run_bass_kernel_spmd(nc, in_maps, core_ids=list(range(NCORES)))
    left = np.concatenate([r["left"] for r in res.results], axis=2)
    right = np.concatenate([r["right"] for r in res.results], axis=2)
    return left, right



# revision 17
# speedup vs baseline: 1.9625x; 1.9625x over previous
"""Dual-pixel depth-merge (forward splat) kernel for Trainium2, 8 NeuronCores.

Math: for integer pixel grid x, the reference computes pos = fl(x +- depth)
(f32-rounded), x0 = floor(pos), f = pos - x0. Define the per-view fractional
offsets
    v_l[i] = fl(i + depth[i]) - i   (exact f32 subtraction, in [0, 8])
    v_r[i] = i - fl(i - depth[i])   (exact f32 subtraction, in [0, 8])
Then each view's splat is a 9-tap shifted weighted sum with hat weights
    W_d = relu(1 - |v - d|), d = 0..8:
    count[j] = sum_d W_d[j-+d]      acc[c,j] = sum_d (W_d*img_c)[j-+d]
    out = acc / max(count, eps)
The kernel carries NEGATED weights w' = min(|v-d|, 1) - 1 = -W_d (one dual-op
tensor_scalar after |v-d|), so acc' = -acc and count' = -count; the final
reciprocal uses count' - eps = -(count + eps), and acc' * rc' = +out. v is
computed in f32 (bit-exact with the reference's rounding of x +- depth, which
pins where count == 0); weights/products run in bf16 (2x DVE tensor_tensor,
4x tensor_scalar; half the DMA bytes).

The 9-tap shifted accumulation runs on the otherwise-idle TensorEngine: for
each 128-column output chunk, 9 identity matmuls (lhsT = I) accumulate the
shifted product slices into one PSUM bank in f32 (start on tap 0, stop on tap
8). ACT copies each finished bank back to SBUF as bf16. ACT also computes
|v - d| for most taps; DVE computes the second weight op (4x tensor_scalar)
and its column share of products; GPSIMD (fused scalar_tensor_tensor, 1.39
ns/elem) takes the rest.

Sharding: pure data parallel over h (the scatter is along w only) - core m
takes h rows [m*128, (m+1)*128) for all batches. No halo, no communication.
"""

import numpy as np
import ml_dtypes

import concourse.bacc as bacc
import concourse.bass as bass
import concourse.mybir as mybir
import concourse.tile as tile
from concourse.bass_utils import run_bass_kernel_spmd

B, C, H, W = 4, 3, 1024, 1024
NCORES = 8
HS = H // NCORES  # 128 h-rows per core
NTAP = 9
CK = 128          # output chunk width = one PSUM bank (4ch * 128 * f32 = 2KB)
NCHUNK = W // CK
F32 = mybir.dt.float32
BF16 = mybir.dt.bfloat16
EPS = 1e-20

A = mybir.AluOpType
_ADD = A.add
_SUB = A.subtract
_MUL = A.mult
_MIN = A.min
_ABSMAX = A.abs_max
_BYPASS = A.bypass
_EQ = A.is_equal
_ABS = mybir.ActivationFunctionType.Abs
_COPY = mybir.ActivationFunctionType.Copy


def _chan(t4, c0, n, col0=0, ncol=W, bc=False):
    """View channels [c0, c0+n) and columns [col0, col0+ncol) of a [HS, ch, W]
    tile as an AP; bc=True instead broadcasts channel c0 n times (step-0 dim).
    n=1 (and not bc) yields a flat [HS, ncol] view. Resolves the tile to a
    sliced AP first so pool-buffer rotation is accounted for."""
    ap = t4[:] if not isinstance(t4, bass.AP) else t4
    a = ap.ap
    pstep = a[0][0]
    w = a[-1][1]
    dims = [[pstep, a[0][1]]]
    if bc or n > 1:
        dims.append([0 if bc else w, n])
    dims.append([1, ncol])
    return bass.AP(tensor=ap.tensor, offset=ap.offset + c0 * w + col0, ap=dims)


CFG = {
    "io": 2, "p": 5, "acc": 2, "v": 2, "t": 3,
    # Column splits (DVE gets [0, s), GPSIMD [s, W)).
    "sp": 780,   # products
    "so": 780,   # final normalize mul
    # Number of taps whose |v-d| runs on ACT (rest on DVE tensor_scalar).
    "act_taps": 9,
}


def build_program(cfg: dict | None = None) -> bass.Bass:
    cfg = {**CFG, **(cfg or {})}
    nc = bacc.Bacc()
    image = nc.dram_tensor("image", [B, C, HS, W], BF16, kind="ExternalInput")
    depth = nc.dram_tensor("depth", [B, HS, W], F32, kind="ExternalInput")
    left = nc.dram_tensor("left", [B, C, HS, W], BF16, kind="ExternalOutput")
    right = nc.dram_tensor("right", [B, C, HS, W], BF16, kind="ExternalOutput")

    sp, so = cfg["sp"], cfg["so"]

    with tile.TileContext(nc) as tc:
        with (
            tc.tile_pool(name="consts", bufs=1) as c_pool,
            tc.tile_pool(name="io", bufs=cfg["io"]) as io_pool,
            tc.tile_pool(name="voff", bufs=cfg["v"]) as v_pool,
            tc.tile_pool(name="tmp", bufs=cfg["t"]) as t_pool,
            tc.tile_pool(name="prod", bufs=cfg["p"]) as p_pool,
            tc.tile_pool(name="accs", bufs=cfg["acc"]) as acc_pool,
            tc.psum_pool(name="ps", bufs=1) as ps_pool,
        ):
            iota_i = t_pool.tile([HS, W], mybir.dt.int32, tag="ti")
            nc.gpsimd.iota(iota_i[:], [[1, W]], channel_multiplier=0)
            iota = c_pool.tile([HS, W], F32, tag="iota")
            nc.vector.tensor_copy(iota[:], iota_i[:])
            # Identity (bf16) for the PE shift-accumulate: I[p, c] = (c == p).
            pidx_i = t_pool.tile([HS, 1], mybir.dt.int32, tag="pi")
            nc.gpsimd.iota(pidx_i[:], [[0, 1]], channel_multiplier=1)
            pidx = c_pool.tile([HS, 1], F32, tag="pidx")
            nc.vector.tensor_copy(pidx[:], pidx_i[:])
            ident = c_pool.tile([HS, CK], BF16, tag="ident")
            nc.vector.tensor_scalar(ident[:], iota[:, 0:CK], pidx[:], None, _EQ)
            # Per-tap ACT bias constants (-d).
            negd = c_pool.tile([HS, NTAP], F32, tag="negd")
            for d in range(NTAP):
                nc.vector.memset(negd[:, d : d + 1], -float(d))

            for b in range(B):
                dep = io_pool.tile([HS, W], F32, tag="dep")
                nc.sync.dma_start(out=dep[:], in_=depth[b])
                img = io_pool.tile([HS, C, W], BF16, tag="img")
                nc.sync.dma_start(out=img[:], in_=image[b].transpose([1, 0, 2]))

                # Exact per-view fractional offsets (f32, matches reference
                # rounding; the second subtraction is exact).
                vl = v_pool.tile([HS, W], F32, tag="vl")
                vr = v_pool.tile([HS, W], F32, tag="vr")
                s = t_pool.tile([HS, W], F32, tag="s")
                nc.vector.tensor_tensor(s[:], dep[:], iota[:], _ADD)
                nc.vector.tensor_tensor(vl[:], s[:], iota[:], _SUB)
                s2 = t_pool.tile([HS, W], F32, tag="s")
                nc.vector.tensor_tensor(s2[:], iota[:], dep[:], _SUB)
                nc.vector.tensor_tensor(vr[:], iota[:], s2[:], _SUB)

                views = (("l", vl), ("r", vr))
                acc_l = acc_pool.tile([HS, 3, W], BF16, tag="al")
                acc_r = acc_pool.tile([HS, 3, W], BF16, tag="ar")
                accs = {"l": acc_l, "r": acc_r}

                for view, v in views:
                    acc = accs[view]
                    pss = []
                    for m in range(NCHUNK):
                        pst = ps_pool.tile([HS, 4, CK], F32, tag=f"ps{m}")
                        pss.append(pst)
                    for d in range(NTAP):
                        # p4 channel 0 = w' = min(|v-d|,1)-1 (negated hat);
                        # channels 1:4 = w' * img.
                        p4 = p_pool.tile([HS, 4, W], BF16, tag="p")
                        td = t_pool.tile([HS, W], F32, tag="t")
                        if d < cfg["act_taps"]:
                            nc.scalar.activation(td[:], v[:], _ABS, bias=negd[:, d : d + 1], scale=1.0)
                        else:
                            nc.vector.tensor_scalar(td[:], v[:], float(d), 0.0, _SUB, _ABSMAX)
                        nc.vector.tensor_scalar(_chan(p4, 0, 1), td[:], 1.0, 0.0, _SUB, _MIN)
                        nc.vector.tensor_tensor(
                            p4[:, 1:4, 0:sp], _chan(p4, 0, 3, 0, sp, bc=True), img[:, :, 0:sp], _MUL
                        )
                        nc.gpsimd.tensor_tensor(
                            p4[:, 1:4, sp:W], _chan(p4, 0, 3, sp, W - sp, bc=True),
                            img[:, :, sp:W], _MUL,
                        )
                        # PE shift-accumulate: chunk m sums shifted tap slices
                        # into its PSUM bank (f32).
                        for m in range(NCHUNK):
                            pst = pss[m]
                            lo = m * CK
                            if view == "l":
                                # out[j] += p[j - d], j in [lo, lo+CK)
                                src = lo - d
                                o0 = max(0, -src)      # skipped leading cols
                                n = CK - o0
                                out_ap = pst[:, :, o0:CK]
                                rhs = _chan(p4, 0, 4, src + o0, n)
                            else:
                                # out[j] += p[j + d]
                                src = lo + d
                                n = min(CK, W - src)
                                out_ap = pst[:, :, 0:n]
                                rhs = _chan(p4, 0, 4, src, n)
                            nc.tensor.matmul(
                                out_ap, ident[:], rhs,
                                start=(d == 0), stop=(d == NTAP - 1),
                            )

                    # PSUM -> SBUF on ACT: img channels to bf16 acc,
                    # count channel to f32 (with the -eps bias folded in).
                    cnt = t_pool.tile([HS, W], F32, tag=f"cn{view}")
                    for m in range(NCHUNK):
                        nc.scalar.activation(
                            acc[:, :, m * CK : (m + 1) * CK], pss[m][:, 1:4, :],
                            _COPY, bias=0.0, scale=1.0,
                        )
                        nc.scalar.activation(
                            cnt[:, m * CK : (m + 1) * CK], _chan(pss[m], 0, 1, 0, CK),
                            _COPY, bias=-EPS, scale=1.0,
                        )
                    # rc' = 1/(count' - eps) = -1/(count + eps)
                    rcf = t_pool.tile([HS, W], F32, tag=f"rf{view}")
                    nc.vector.reciprocal_approx_fast(out=rcf[:], in_=cnt[:])
                    rc = t_pool.tile([HS, W], BF16, tag=f"rc{view}")
                    nc.scalar.activation(rc[:], rcf[:], _COPY, bias=0.0, scale=1.0)
                    # out = acc' * rc' (in place), split DVE/GPSIMD
                    nc.vector.tensor_tensor(
                        acc[:, :, 0:so], acc[:, :, 0:so], _chan(rc, 0, 3, 0, so, bc=True), _MUL
                    )
                    nc.gpsimd.tensor_tensor(
                        acc[:, :, so:W], acc[:, :, so:W],
                        _chan(rc, 0, 3, so, W - so, bc=True), _MUL,
                    )
                    dst = left if view == "l" else right
                    nc.sync.dma_start(out=dst[b].transpose([1, 0, 2]), in_=acc[:])
    nc.compile()
    return nc


_NC_CACHE = None


def _get_program():
    global _NC_CACHE
    if _NC_CACHE is None:
        _NC_CACHE = build_program()
    return _NC_CACHE


def kernel(image: np.ndarray, depth: np.ndarray):
    image = np.ascontiguousarray(image, dtype=np.float32)
    depth = np.ascontiguousarray(depth, dtype=np.float32)
    assert image.shape == (B, C, H, W) and depth.shape == (B, H, W)
    image16 = image.astype(ml_dtypes.bfloat16)

    nc = _get_program()
    in_maps = []
    for m in range(NCORES):
        sl = slice(m * HS, (m + 1) * HS)
        in_maps.append(
            {
                "image": np.ascontiguousarray(image16[:, :, sl, :]),
                "depth": np.ascontiguousarray(depth[:, sl, :]),
            }
        )
    # The axon-tunneled devices occasionally come up in a transient
    # unrecoverable/desynced state and recover on the next attempt - retry
    # once before giving up.
    try:
        res = run_bass_kernel_spmd(nc, in_maps, core_ids=list(range(NCORES)))
    except Exception:
        import time as _time

        _time.sleep(5.0)
        res = run_bass_kernel_spmd(nc, in_maps, core_ids=list(range(NCORES)))
    left = np.concatenate(
        [np.asarray(r["left"]).astype(np.float32) for r in res.results], axis=2
    )
    right = np.concatenate(
        [np.asarray(r["right"]).astype(np.float32) for r in res.results], axis=2
    )
    return left, right


# revision 20
# speedup vs baseline: 1.9719x; 1.0048x over previous
"""Dual-pixel depth-merge (forward splat) kernel for Trainium2, 8 NeuronCores.

Math: for integer pixel grid x, the reference computes pos = fl(x +- depth)
(f32-rounded), x0 = floor(pos), f = pos - x0. Define the per-view fractional
offsets
    v_l[i] = fl(i + depth[i]) - i   (exact f32 subtraction, in [0, 8])
    v_r[i] = i - fl(i - depth[i])   (exact f32 subtraction, in [0, 8])
Then each view's splat is a 9-tap shifted weighted sum with hat weights
    W_d = relu(1 - |v - d|), d = 0..8:
    count[j] = sum_d W_d[j-+d]      acc[c,j] = sum_d (W_d*img_c)[j-+d]
    out = acc / max(count, eps)
The kernel carries NEGATED weights w' = min(|v-d|, 1) - 1 = -W_d (one dual-op
tensor_scalar after |v-d|), so acc' = -acc and count' = -count; the final
reciprocal uses count' - eps = -(count + eps), and acc' * rc' = +out. v is
computed in f32 (bit-exact with the reference's rounding of x +- depth, which
pins where count == 0); weights/products run in bf16 (2x DVE tensor_tensor,
4x tensor_scalar; half the DMA bytes).

The 9-tap shifted accumulation runs on the otherwise-idle TensorEngine: for
each 128-column output chunk, 9 identity matmuls (lhsT = I) accumulate the
shifted product slices into one PSUM bank in f32 (start on tap 0, stop on tap
8). ACT copies each finished bank back to SBUF as bf16. ACT also computes
|v - d| for most taps; DVE computes the second weight op (4x tensor_scalar)
and its column share of products; GPSIMD (fused scalar_tensor_tensor, 1.39
ns/elem) takes the rest.

Sharding: pure data parallel over h (the scatter is along w only) - core m
takes h rows [m*128, (m+1)*128) for all batches. No halo, no communication.
"""

import numpy as np
import ml_dtypes

import concourse.bacc as bacc
import concourse.bass as bass
import concourse.mybir as mybir
import concourse.tile as tile
from concourse.bass_utils import run_bass_kernel_spmd

B, C, H, W = 4, 3, 1024, 1024
NCORES = 8
HS = H // NCORES  # 128 h-rows per core
NTAP = 9
CK = 128          # output chunk width = one PSUM bank (4ch * 128 * f32 = 2KB)
NCHUNK = W // CK
F32 = mybir.dt.float32
BF16 = mybir.dt.bfloat16
EPS = 1e-20

A = mybir.AluOpType
_ADD = A.add
_SUB = A.subtract
_MUL = A.mult
_MIN = A.min
_ABSMAX = A.abs_max
_BYPASS = A.bypass
_EQ = A.is_equal
_ABS = mybir.ActivationFunctionType.Abs
_COPY = mybir.ActivationFunctionType.Copy
_RELU = mybir.ActivationFunctionType.Relu


def _chan(t4, c0, n, col0=0, ncol=W, bc=False):
    """View channels [c0, c0+n) and columns [col0, col0+ncol) of a [HS, ch, W]
    tile as an AP; bc=True instead broadcasts channel c0 n times (step-0 dim).
    n=1 (and not bc) yields a flat [HS, ncol] view. Resolves the tile to a
    sliced AP first so pool-buffer rotation is accounted for."""
    ap = t4[:] if not isinstance(t4, bass.AP) else t4
    a = ap.ap
    pstep = a[0][0]
    w = a[-1][1]
    dims = [[pstep, a[0][1]]]
    if bc or n > 1:
        dims.append([0 if bc else w, n])
    dims.append([1, ncol])
    return bass.AP(tensor=ap.tensor, offset=ap.offset + c0 * w + col0, ap=dims)


CFG = {
    "io": 2, "dio": 4, "p": 5, "acc": 2, "v": 2, "t": 3,
    # Column splits (DVE gets [0, s), GPSIMD [s, W)).
    "sp": 731,   # products
    "so": 731,   # final normalize mul
    # Number of taps whose |v-d| runs on ACT (rest on DVE tensor_scalar).
    "act_taps": 9,
    # Taps (per view) whose second weight op runs on ACT as Relu(1-t), giving
    # +W; those taps use the negated identity in the PE accumulate.
    "a2": {"l": 1, "r": 0},
}


def build_program(cfg: dict | None = None) -> bass.Bass:
    cfg = {**CFG, **(cfg or {})}
    nc = bacc.Bacc()
    image = nc.dram_tensor("image", [B, C, HS, W], BF16, kind="ExternalInput")
    depth = nc.dram_tensor("depth", [B, HS, W], F32, kind="ExternalInput")
    left = nc.dram_tensor("left", [B, C, HS, W], BF16, kind="ExternalOutput")
    right = nc.dram_tensor("right", [B, C, HS, W], BF16, kind="ExternalOutput")

    sp, so = cfg["sp"], cfg["so"]

    with tile.TileContext(nc) as tc:
        with (
            tc.tile_pool(name="consts", bufs=1) as c_pool,
            tc.tile_pool(name="io", bufs=cfg["io"]) as io_pool,
            tc.tile_pool(name="dio", bufs=cfg["dio"]) as d_pool,
            tc.tile_pool(name="voff", bufs=cfg["v"]) as v_pool,
            tc.tile_pool(name="tmp", bufs=cfg["t"]) as t_pool,
            tc.tile_pool(name="tmp1", bufs=1) as t1_pool,
            tc.tile_pool(name="tmp2", bufs=2) as t2_pool,
            tc.tile_pool(name="prod", bufs=cfg["p"]) as p_pool,
            tc.tile_pool(name="accs", bufs=cfg["acc"]) as acc_pool,
            tc.psum_pool(name="ps", bufs=1) as ps_pool,
        ):
            iota_i = t1_pool.tile([HS, W], mybir.dt.int32, tag="ti")
            nc.gpsimd.iota(iota_i[:], [[1, W]], channel_multiplier=0)
            iota = c_pool.tile([HS, W], F32, tag="iota")
            nc.vector.tensor_copy(iota[:], iota_i[:])
            # Identity (bf16) for the PE shift-accumulate: I[p, c] = (c == p).
            pidx_i = t1_pool.tile([HS, 1], mybir.dt.int32, tag="pi")
            nc.gpsimd.iota(pidx_i[:], [[0, 1]], channel_multiplier=1)
            pidx = c_pool.tile([HS, 1], F32, tag="pidx")
            nc.vector.tensor_copy(pidx[:], pidx_i[:])
            ident = c_pool.tile([HS, CK], BF16, tag="ident")
            nc.vector.tensor_scalar(ident[:], iota[:, 0:CK], pidx[:], None, _EQ)
            identn = c_pool.tile([HS, CK], BF16, tag="identn")
            nc.vector.tensor_scalar(identn[:], iota[:, 0:CK], pidx[:], -1.0, _EQ, _MUL)
            # Per-tap ACT bias constants (-d).
            negd = c_pool.tile([HS, NTAP], F32, tag="negd")
            for d in range(NTAP):
                nc.vector.memset(negd[:, d : d + 1], -float(d))

            for b in range(B):
                dep = d_pool.tile([HS, W], F32, tag="dep")
                nc.sync.dma_start(out=dep[:], in_=depth[b])
                img = io_pool.tile([HS, C, W], BF16, tag="img")
                nc.sync.dma_start(out=img[:], in_=image[b].transpose([1, 0, 2]))

                # Exact per-view fractional offsets (f32, matches reference
                # rounding; the second subtraction is exact).
                vl = v_pool.tile([HS, W], F32, tag="vl")
                vr = v_pool.tile([HS, W], F32, tag="vr")
                s = t2_pool.tile([HS, W], F32, tag="s")
                nc.vector.tensor_tensor(s[:], dep[:], iota[:], _ADD)
                nc.vector.tensor_tensor(vl[:], s[:], iota[:], _SUB)
                s2 = t2_pool.tile([HS, W], F32, tag="s")
                nc.vector.tensor_tensor(s2[:], iota[:], dep[:], _SUB)
                nc.vector.tensor_tensor(vr[:], iota[:], s2[:], _SUB)

                views = (("l", vl), ("r", vr))
                acc_l = acc_pool.tile([HS, 3, W], BF16, tag="al")
                acc_r = acc_pool.tile([HS, 3, W], BF16, tag="ar")
                accs = {"l": acc_l, "r": acc_r}

                for view, v in views:
                    acc = accs[view]
                    pss = []
                    for m in range(NCHUNK):
                        pst = ps_pool.tile([HS, 4, CK], F32, tag=f"ps{m}")
                        pss.append(pst)
                    for d in range(NTAP):
                        # p4 channel 0 = w' = min(|v-d|,1)-1 (negated hat);
                        # channels 1:4 = w' * img.
                        p4 = p_pool.tile([HS, 4, W], BF16, tag="p")
                        td = t_pool.tile([HS, W], F32, tag="t")
                        nc.scalar.activation(td[:], v[:], _ABS, bias=negd[:, d : d + 1], scale=1.0)
                        pos_w = d < cfg["a2"][view]
                        if pos_w:
                            nc.scalar.activation(_chan(p4, 0, 1), td[:], _RELU, bias=1.0, scale=-1.0)
                        else:
                            nc.vector.tensor_scalar(_chan(p4, 0, 1), td[:], 1.0, 0.0, _SUB, _MIN)
                        nc.vector.tensor_tensor(
                            p4[:, 1:4, 0:sp], _chan(p4, 0, 3, 0, sp, bc=True), img[:, :, 0:sp], _MUL
                        )
                        nc.gpsimd.tensor_tensor(
                            p4[:, 1:4, sp:W], _chan(p4, 0, 3, sp, W - sp, bc=True),
                            img[:, :, sp:W], _MUL,
                        )
                        # PE shift-accumulate: chunk m sums shifted tap slices
                        # into its PSUM bank (f32).
                        for m in range(NCHUNK):
                            pst = pss[m]
                            lo = m * CK
                            if view == "l":
                                # out[j] += p[j - d], j in [lo, lo+CK)
                                src = lo - d
                                o0 = max(0, -src)      # skipped leading cols
                                n = CK - o0
                                out_ap = pst[:, :, o0:CK]
                                rhs = _chan(p4, 0, 4, src + o0, n)
                            else:
                                # out[j] += p[j + d]
                                src = lo + d
                                n = min(CK, W - src)
                                out_ap = pst[:, :, 0:n]
                                rhs = _chan(p4, 0, 4, src, n)
                            nc.tensor.matmul(
                                out_ap, (identn if pos_w else ident)[:], rhs,
                                start=(d == 0), stop=(d == NTAP - 1),
                            )

                    # PSUM -> SBUF on ACT: img channels to bf16 acc,
                    # count channel to f32 (with the -eps bias folded in).
                    cnt = t2_pool.tile([HS, W], F32, tag=f"cn{view}")
                    for m in range(NCHUNK):
                        nc.scalar.activation(
                            acc[:, :, m * CK : (m + 1) * CK], pss[m][:, 1:4, :],
                            _COPY, bias=0.0, scale=1.0,
                        )
                        nc.scalar.activation(
                            cnt[:, m * CK : (m + 1) * CK], _chan(pss[m], 0, 1, 0, CK),
                            _COPY, bias=-EPS, scale=1.0,
                        )
                    # rc' = 1/(count' - eps) = -1/(count + eps)
                    rcf = t2_pool.tile([HS, W], F32, tag=f"rf{view}")
                    nc.vector.reciprocal_approx_fast(out=rcf[:], in_=cnt[:])
                    rc = t2_pool.tile([HS, W], BF16, tag=f"rc{view}")
                    nc.scalar.activation(rc[:], rcf[:], _COPY, bias=0.0, scale=1.0)
                    # out = acc' * rc' (in place), split DVE/GPSIMD
                    nc.vector.tensor_tensor(
                        acc[:, :, 0:so], acc[:, :, 0:so], _chan(rc, 0, 3, 0, so, bc=True), _MUL
                    )
                    nc.gpsimd.tensor_tensor(
                        acc[:, :, so:W], acc[:, :, so:W],
                        _chan(rc, 0, 3, so, W - so, bc=True), _MUL,
                    )
                    dst = left if view == "l" else right
                    nc.sync.dma_start(out=dst[b].transpose([1, 0, 2]), in_=acc[:])
    nc.compile()
    return nc


_NC_CACHE = None


def _get_program():
    global _NC_CACHE
    if _NC_CACHE is None:
        _NC_CACHE = build_program()
    return _NC_CACHE


def kernel(image: np.ndarray, depth: np.ndarray):
    image = np.ascontiguousarray(image, dtype=np.float32)
    depth = np.ascontiguousarray(depth, dtype=np.float32)
    assert image.shape == (B, C, H, W) and depth.shape == (B, H, W)
    image16 = image.astype(ml_dtypes.bfloat16)

    nc = _get_program()
    in_maps = []
    for m in range(NCORES):
        sl = slice(m * HS, (m + 1) * HS)
        in_maps.append(
            {
                "image": np.ascontiguousarray(image16[:, :, sl, :]),
                "depth": np.ascontiguousarray(depth[:, sl, :]),
            }
        )
    # The axon-tunneled devices occasionally come up in a transient
    # unrecoverable/desynced state and recover on the next attempt - retry
    # once before giving up.
    try:
        res = run_bass_kernel_spmd(nc, in_maps, core_ids=list(range(NCORES)))
    except Exception:
        import time as _time

        _time.sleep(5.0)
        res = run_bass_kernel_spmd(nc, in_maps, core_ids=list(range(NCORES)))
    left = np.concatenate(
        [np.asarray(r["left"]).astype(np.float32) for r in res.results], axis=2
    )
    right = np.concatenate(
        [np.asarray(r["right"]).astype(np.float32) for r in res.results], axis=2
    )
    return left, right


# revision 21
# speedup vs baseline: 2.0128x; 1.0207x over previous
"""Dual-pixel depth-merge (forward splat) kernel for Trainium2, 8 NeuronCores.

Math: for integer pixel grid x, the reference computes pos = fl(x +- depth)
(f32-rounded), x0 = floor(pos), f = pos - x0. Define the per-view fractional
offsets
    v_l[i] = fl(i + depth[i]) - i   (exact f32 subtraction, in [0, 8])
    v_r[i] = i - fl(i - depth[i])   (exact f32 subtraction, in [0, 8])
Then each view's splat is a 9-tap shifted weighted sum with hat weights
    W_d = relu(1 - |v - d|), d = 0..8:
    count[j] = sum_d W_d[j-+d]      acc[c,j] = sum_d (W_d*img_c)[j-+d]
    out = acc / max(count, eps)
The kernel carries NEGATED weights w' = min(|v-d|, 1) - 1 = -W_d (one dual-op
tensor_scalar after |v-d|), so acc' = -acc and count' = -count; the final
reciprocal uses count' - eps = -(count + eps), and acc' * rc' = +out. v is
computed in f32 (bit-exact with the reference's rounding of x +- depth, which
pins where count == 0); weights/products run in bf16 (2x DVE tensor_tensor,
4x tensor_scalar; half the DMA bytes).

The 9-tap shifted accumulation runs on the otherwise-idle TensorEngine: for
each 128-column output chunk, 9 identity matmuls (lhsT = I) accumulate the
shifted product slices into one PSUM bank in f32 (start on tap 0, stop on tap
8). ACT copies each finished bank back to SBUF as bf16. ACT also computes
|v - d| for most taps; DVE computes the second weight op (4x tensor_scalar)
and its column share of products; GPSIMD (fused scalar_tensor_tensor, 1.39
ns/elem) takes the rest.

Sharding: pure data parallel over h (the scatter is along w only) - core m
takes h rows [m*128, (m+1)*128) for all batches. No halo, no communication.
"""

import numpy as np
import ml_dtypes

import concourse.bacc as bacc
import concourse.bass as bass
import concourse.mybir as mybir
import concourse.tile as tile
from concourse.bass_utils import run_bass_kernel_spmd

B, C, H, W = 4, 3, 1024, 1024
NCORES = 8
HS = H // NCORES  # 128 h-rows per core
NTAP = 9
CK = 128          # output chunk width = one PSUM bank (4ch * 128 * f32 = 2KB)
NCHUNK = W // CK
F32 = mybir.dt.float32
BF16 = mybir.dt.bfloat16
EPS = 1e-20

A = mybir.AluOpType
_ADD = A.add
_SUB = A.subtract
_MUL = A.mult
_MIN = A.min
_ABSMAX = A.abs_max
_BYPASS = A.bypass
_EQ = A.is_equal
_ABS = mybir.ActivationFunctionType.Abs
_COPY = mybir.ActivationFunctionType.Copy
_RELU = mybir.ActivationFunctionType.Relu


def _chan(t4, c0, n, col0=0, ncol=W, bc=False):
    """View channels [c0, c0+n) and columns [col0, col0+ncol) of a [HS, ch, W]
    tile as an AP; bc=True instead broadcasts channel c0 n times (step-0 dim).
    n=1 (and not bc) yields a flat [HS, ncol] view. Resolves the tile to a
    sliced AP first so pool-buffer rotation is accounted for."""
    ap = t4[:] if not isinstance(t4, bass.AP) else t4
    a = ap.ap
    pstep = a[0][0]
    w = a[-1][1]
    dims = [[pstep, a[0][1]]]
    if bc or n > 1:
        dims.append([0 if bc else w, n])
    dims.append([1, ncol])
    return bass.AP(tensor=ap.tensor, offset=ap.offset + c0 * w + col0, ap=dims)


CFG = {
    "io": 2, "dio": 4, "p": 5, "acc": 2, "v": 2, "t": 3,
    # Column splits (DVE gets [0, s), GPSIMD [s, W)).
    "sp": 755,   # products
    "so": 755,   # final normalize mul
    # Number of taps whose |v-d| runs on ACT (rest on DVE tensor_scalar).
    "act_taps": 9,
    # Taps (per view) whose second weight op runs on ACT as Relu(1-t), giving
    # +W; those taps use the negated identity in the PE accumulate.
    "a2": {"l": 1, "r": 0},
}


def build_program(cfg: dict | None = None) -> bass.Bass:
    cfg = {**CFG, **(cfg or {})}
    nc = bacc.Bacc()
    image = nc.dram_tensor("image", [B, C, HS, W], BF16, kind="ExternalInput")
    depth = nc.dram_tensor("depth", [B, HS, W], F32, kind="ExternalInput")
    left = nc.dram_tensor("left", [B, C, HS, W], BF16, kind="ExternalOutput")
    right = nc.dram_tensor("right", [B, C, HS, W], BF16, kind="ExternalOutput")

    sp, so = cfg["sp"], cfg["so"]

    with tile.TileContext(nc) as tc:
        with (
            tc.tile_pool(name="consts", bufs=1) as c_pool,
            tc.tile_pool(name="io", bufs=cfg["io"]) as io_pool,
            tc.tile_pool(name="dio", bufs=cfg["dio"]) as d_pool,
            tc.tile_pool(name="voff", bufs=cfg["v"]) as v_pool,
            tc.tile_pool(name="tmp", bufs=cfg["t"]) as t_pool,
            tc.tile_pool(name="tmp1", bufs=1) as t1_pool,
            tc.tile_pool(name="tmp2", bufs=2) as t2_pool,
            tc.tile_pool(name="prod", bufs=cfg["p"]) as p_pool,
            tc.tile_pool(name="accs", bufs=cfg["acc"]) as acc_pool,
            tc.psum_pool(name="ps", bufs=1) as ps_pool,
        ):
            iota_i = t1_pool.tile([HS, W], mybir.dt.int32, tag="ti")
            nc.gpsimd.iota(iota_i[:], [[1, W]], channel_multiplier=0)
            iota = c_pool.tile([HS, W], F32, tag="iota")
            nc.vector.tensor_copy(iota[:], iota_i[:])
            # Identity (bf16) for the PE shift-accumulate: I[p, c] = (c == p).
            pidx_i = t1_pool.tile([HS, 1], mybir.dt.int32, tag="pi")
            nc.gpsimd.iota(pidx_i[:], [[0, 1]], channel_multiplier=1)
            pidx = c_pool.tile([HS, 1], F32, tag="pidx")
            nc.vector.tensor_copy(pidx[:], pidx_i[:])
            ident = c_pool.tile([HS, CK], BF16, tag="ident")
            nc.vector.tensor_scalar(ident[:], iota[:, 0:CK], pidx[:], None, _EQ)
            identn = c_pool.tile([HS, CK], BF16, tag="identn")
            nc.vector.tensor_scalar(identn[:], iota[:, 0:CK], pidx[:], -1.0, _EQ, _MUL)
            # Per-tap ACT bias constants (-d).
            negd = c_pool.tile([HS, NTAP], F32, tag="negd")
            for d in range(NTAP):
                nc.vector.memset(negd[:, d : d + 1], -float(d))

            for b in range(B):
                dep = d_pool.tile([HS, W], F32, tag="dep")
                nc.sync.dma_start(out=dep[:], in_=depth[b])
                img = io_pool.tile([HS, C, W], BF16, tag="img")
                nc.sync.dma_start(out=img[:], in_=image[b].transpose([1, 0, 2]))

                # Exact per-view fractional offsets (f32, matches reference
                # rounding; the second subtraction is exact).
                vl = v_pool.tile([HS, W], F32, tag="vl")
                vr = v_pool.tile([HS, W], F32, tag="vr")
                s = t2_pool.tile([HS, W], F32, tag="s")
                nc.vector.tensor_tensor(s[:], dep[:], iota[:], _ADD)
                nc.vector.tensor_tensor(vl[:], s[:], iota[:], _SUB)
                s2 = t2_pool.tile([HS, W], F32, tag="s")
                nc.vector.tensor_tensor(s2[:], iota[:], dep[:], _SUB)
                nc.vector.tensor_tensor(vr[:], iota[:], s2[:], _SUB)

                views = (("l", vl), ("r", vr))
                acc_l = acc_pool.tile([HS, 3, W], BF16, tag="al")
                acc_r = acc_pool.tile([HS, 3, W], BF16, tag="ar")
                accs = {"l": acc_l, "r": acc_r}

                for view, v in views:
                    acc = accs[view]
                    pss = []
                    for m in range(NCHUNK):
                        pst = ps_pool.tile([HS, 4, CK], F32, tag=f"ps{m}")
                        pss.append(pst)
                    for d in range(NTAP):
                        # p4 channel 0 = w' = min(|v-d|,1)-1 (negated hat);
                        # channels 1:4 = w' * img.
                        p4 = p_pool.tile([HS, 4, W], BF16, tag="p")
                        td = t_pool.tile([HS, W], F32, tag="t")
                        nc.scalar.activation(td[:], v[:], _ABS, bias=negd[:, d : d + 1], scale=1.0)
                        pos_w = d < cfg["a2"][view]
                        if pos_w:
                            nc.scalar.activation(_chan(p4, 0, 1), td[:], _RELU, bias=1.0, scale=-1.0)
                        else:
                            nc.vector.tensor_scalar(_chan(p4, 0, 1), td[:], 1.0, 0.0, _SUB, _MIN)
                        nc.vector.tensor_tensor(
                            p4[:, 1:4, 0:sp], _chan(p4, 0, 3, 0, sp, bc=True), img[:, :, 0:sp], _MUL
                        )
                        nc.gpsimd.tensor_tensor(
                            p4[:, 1:4, sp:W], _chan(p4, 0, 3, sp, W - sp, bc=True),
                            img[:, :, sp:W], _MUL,
                        )
                        # PE shift-accumulate: chunk m sums shifted tap slices
                        # into its PSUM bank (f32).
                        for m in range(NCHUNK):
                            pst = pss[m]
                            lo = m * CK
                            if view == "l":
                                # out[j] += p[j - d], j in [lo, lo+CK)
                                src = lo - d
                                o0 = max(0, -src)      # skipped leading cols
                                n = CK - o0
                                out_ap = pst[:, :, o0:CK]
                                rhs = _chan(p4, 0, 4, src + o0, n)
                            else:
                                # out[j] += p[j + d]
                                src = lo + d
                                n = min(CK, W - src)
                                out_ap = pst[:, :, 0:n]
                                rhs = _chan(p4, 0, 4, src, n)
                            nc.tensor.matmul(
                                out_ap, (identn if pos_w else ident)[:], rhs,
                                start=(d == 0), stop=(d == NTAP - 1),
                            )

                    # PSUM -> SBUF on ACT: img channels to bf16 acc,
                    # count channel to f32 (with the -eps bias folded in).
                    cnt = t2_pool.tile([HS, W], F32, tag=f"cn{view}")
                    for m in range(NCHUNK):
                        nc.scalar.activation(
                            acc[:, :, m * CK : (m + 1) * CK], pss[m][:, 1:4, :],
                            _COPY, bias=0.0, scale=1.0,
                        )
                        nc.scalar.activation(
                            cnt[:, m * CK : (m + 1) * CK], _chan(pss[m], 0, 1, 0, CK),
                            _COPY, bias=-EPS, scale=1.0,
                        )
                    # rc' = 1/(count' - eps) = -1/(count + eps)
                    rcf = t2_pool.tile([HS, W], F32, tag=f"rf{view}")
                    nc.vector.reciprocal_approx_fast(out=rcf[:], in_=cnt[:])
                    rc = t2_pool.tile([HS, W], BF16, tag=f"rc{view}")
                    nc.scalar.activation(rc[:], rcf[:], _COPY, bias=0.0, scale=1.0)
                    # out = acc' * rc' (in place), split DVE/GPSIMD
                    nc.vector.tensor_tensor(
                        acc[:, :, 0:so], acc[:, :, 0:so], _chan(rc, 0, 3, 0, so, bc=True), _MUL
                    )
                    nc.gpsimd.tensor_tensor(
                        acc[:, :, so:W], acc[:, :, so:W],
                        _chan(rc, 0, 3, so, W - so, bc=True), _MUL,
                    )
                    dst = left if view == "l" else right
                    nc.sync.dma_start(out=dst[b].transpose([1, 0, 2]), in_=acc[:])
    nc.compile()
    return nc


_NC_CACHE = None


def _get_program():
    global _NC_CACHE
    if _NC_CACHE is None:
        _NC_CACHE = build_program()
    return _NC_CACHE


def kernel(image: np.ndarray, depth: np.ndarray):
    image = np.ascontiguousarray(image, dtype=np.float32)
    depth = np.ascontiguousarray(depth, dtype=np.float32)
    assert image.shape == (B, C, H, W) and depth.shape == (B, H, W)
    image16 = image.astype(ml_dtypes.bfloat16)

    nc = _get_program()
    in_maps = []
    for m in range(NCORES):
        sl = slice(m * HS, (m + 1) * HS)
        in_maps.append(
            {
                "image": np.ascontiguousarray(image16[:, :, sl, :]),
                "depth": np.ascontiguousarray(depth[:, sl, :]),
            }
        )
    # The axon-tunneled devices occasionally come up in a transient
    # unrecoverable/desynced state and recover on the next attempt - retry
    # once before giving up.
    try:
        res = run_bass_kernel_spmd(nc, in_maps, core_ids=list(range(NCORES)))
    except Exception:
        import time as _time

        _time.sleep(5.0)
        res = run_bass_kernel_spmd(nc, in_maps, core_ids=list(range(NCORES)))
    left = np.concatenate(
        [np.asarray(r["left"]).astype(np.float32) for r in res.results], axis=2
    )
    right = np.concatenate(
        [np.asarray(r["right"]).astype(np.float32) for r in res.results], axis=2
    )
    return left, right
